# revision 12
# baseline (speedup 1.0000x reference)
"""AFT-Full (Attention Free Transformer) kernel for Trainium2, 8 NeuronCores.

Model (per batch b):
    q = x @ Wq + bq;  k = x @ Wk + bk;  v = x @ Wv + bv
    out[i,d] = sigmoid(q)[i,d] * sum_j exp(B[i,j])*exp(k[j,d])*v[j,d]
                               / sum_j exp(B[i,j])*exp(k[j,d])

Sharding: data-parallel over batch (BS=8 -> 1 batch per core). pos_bias is
replicated (transposed on host so the contraction index j lands on SBUF
partitions). Matmul operands are bf16; accumulation is fp32 in PSUM.

Bias handling (no bias matmuls at all):
  - bk cancels exactly in the num/den ratio -> dropped.
  - bv enters linearly: ekv = ek * (v + bv), added on DVE in phase 1b.
  - bq folds into eq: exp(-(q+bq)) = exp(-q)*exp(-bq), with exp(-bq)
    precomputed on host and broadcast-multiplied on DVE.

Per-core device schedule:
  warmup:   a few dependency-free matmuls on memset tiles raise the PE HAM
            clock gate to full rate while the first DMAs land.
  phase 1a: q-projection (4 K-chunk matmuls into PSUM), eq = exp(-q)*exp(-bq).
  phase 1b: k,v projections; ek = exp(k) (ACT -> bf16), ekv = ek*(v+bv)
            (DVE -> bf16), stored as X = [ekv | ek] per j-chunk.
  phase 2:  per output i-chunk: DMA bf16 tile of exp(B^T) (exp precomputed
            host-side on the replicated tensor), 32 accumulating matmuls
            (num, den), epilogue
            out = num / (den * (1 + eq)) on DVE. Only Exp is ever used on ACT
            so there is exactly one activation-table load.
"""

import os
import sys

import ml_dtypes
import numpy as np

for _p in ("/opt/trn_rl_repo", "/root/.axon_site/_ro/trn_rl_repo"):
    if os.path.isdir(_p) and _p not in sys.path:
        sys.path.insert(0, _p)

import concourse.bass as bass
import concourse.tile as tile
from concourse import bacc, mybir
from concourse.bass_utils import run_bass_kernel_spmd

BS, N, D = 8, 2048, 512
P = 128
NCH = N // P  # 16 sequence chunks
KC = D // P  # 4 contraction chunks for projections
NB = 4  # xT column blocks (of 512) for startup pipelining
NWARM = 10
F32 = mybir.dt.float32
BF16 = mybir.dt.bfloat16
NP_BF16 = ml_dtypes.bfloat16

_NC_CACHE = {}


def _pbcast(ap_1xd, parts):
    """[1, D] dram AP -> partition-broadcast [parts, D] AP for DMA."""
    return bass.AP(
        tensor=ap_1xd.tensor, offset=ap_1xd.offset, ap=[[0, parts], ap_1xd.ap[1]]
    )


def build_nc():
    nc = bacc.Bacc("TRN2", target_bir_lowering=False, debug=False, num_devices=BS)

    xT = nc.dram_tensor("xT", [D, N], BF16, kind="ExternalInput").ap()
    wqkv = nc.dram_tensor("wqkv", [D, 3 * D], BF16, kind="ExternalInput").ap()
    ebq = nc.dram_tensor("ebq", [1, D], F32, kind="ExternalInput").ap()  # exp(-bq)
    bv = nc.dram_tensor("bv", [1, D], F32, kind="ExternalInput").ap()
    ebt = nc.dram_tensor("ebt", [N, N], BF16, kind="ExternalInput").ap()
    out = nc.dram_tensor("out", [N, D], F32, kind="ExternalOutput").ap()

    # B^T viewed as [ji(=partition), jo, i]
    ebt_v = ebt.rearrange("(jo ji) i -> ji jo i", ji=P)

    with tile.TileContext(nc) as tc:
        with (
            tc.tile_pool(name="consts", bufs=1) as consts,
            tc.tile_pool(name="proj", bufs=1) as proj,
            tc.tile_pool(name="xpool", bufs=1) as xpool,
            tc.tile_pool(name="eqpool", bufs=1) as eqpool,
            tc.tile_pool(name="ebpool", bufs=3) as ebpool,
            tc.tile_pool(name="epi", bufs=2) as epi,
            tc.tile_pool(name="psum", bufs=2, space="PSUM") as psum,
        ):
            # ---- PE pre-warm: dependency-free matmuls (uninitialized SBUF
            # reads; results never consumed) raise the HAM clock gate while
            # the first input DMAs are in flight. Alternating PSUM banks keep
            # them back-to-back.
            warm_w = consts.tile([P, P], BF16, tag="warm_w")
            nc.gpsimd.memset(warm_w, 1.0)
            warm_r = consts.tile([P, D], BF16, tag="warm_r")
            nc.vector.memset(warm_r, 1.0)
            warm_ps = psum.tile([P, D], F32, tag="A", bufs=3)
            for w in range(NWARM):
                nc.tensor.matmul(
                    warm_ps, warm_w, warm_r,
                    start=(w == 0), stop=(w == NWARM - 1),
                )

            # weights + input.T, q-columns and first xT block interleaved
            # first so phase 1a's first accumulation group can start ASAP
            w_t = {}
            xt_b = {}

            def _dma_w(proj_i, c):
                w = proj.tile([P, D], BF16, tag=f"w{proj_i}_{c}")
                nc.sync.dma_start(
                    w, wqkv[c * P : (c + 1) * P, proj_i * D : (proj_i + 1) * D]
                )
                w_t[proj_i, c] = w

            def _dma_xt(b, c):
                x = proj.tile([P, N // NB], BF16, tag=f"xt{b}_{c}")
                nc.sync.dma_start(
                    x,
                    xT[c * P : (c + 1) * P, b * (N // NB) : (b + 1) * (N // NB)],
                )
                xt_b[b, c] = x

            # issue order matches consumption order: q chunks first, then
            # the k/v weights needed right after the q loop finishes
            for c in range(KC):
                _dma_w(0, c)
                _dma_xt(0, c)
            for c in range(KC):
                _dma_xt(1, c)
            for c in range(KC):
                _dma_xt(2, c)
            for c in range(KC):
                _dma_w(1, c)
            for c in range(KC):
                _dma_xt(3, c)
            for c in range(KC):
                _dma_w(2, c)
            ebq_bc = consts.tile([P, D], F32, tag="ebq")
            nc.sync.dma_start(ebq_bc, _pbcast(ebq, P))
            bv_bc = consts.tile([P, D], F32, tag="bv")
            nc.sync.dma_start(bv_bc, _pbcast(bv, P))

            def lhs(n, c):
                b, r = divmod(n, NB)
                return xt_b[b, c][:, r * P : (r + 1) * P]

            # ---- phase 1a: q projection, eq = exp(-q)*exp(-bq) ----
            eq_t = []
            for n in range(NCH):
                ps = psum.tile([P, D], F32, tag="A", bufs=3)
                for c in range(KC):
                    nc.tensor.matmul(
                        ps, lhs(n, c), w_t[0, c], start=(c == 0), stop=(c == KC - 1)
                    )
                eq = eqpool.tile([P, D], F32, tag=f"eq{n}")
                nc.scalar.activation(
                    eq, ps, mybir.ActivationFunctionType.Exp, scale=-1.0
                )
                nc.vector.tensor_mul(eq, eq, ebq_bc)
                eq_t.append(eq)

            # ---- phase 1b: k, v projections; X = [ekv | ek] ----
            x_t = []
            for n in range(NCH):
                psk = psum.tile([P, D], F32, tag="A", bufs=3)
                psv = psum.tile([P, D], F32, tag="B", bufs=3)
                for c in range(KC):
                    nc.tensor.matmul(
                        psk, lhs(n, c), w_t[1, c], start=(c == 0), stop=(c == KC - 1)
                    )
                    nc.tensor.matmul(
                        psv, lhs(n, c), w_t[2, c], start=(c == 0), stop=(c == KC - 1)
                    )
                xt_tile = xpool.tile([P, 2 * D], BF16, tag=f"X{n}")
                nc.scalar.activation(
                    xt_tile[:, D : 2 * D], psk, mybir.ActivationFunctionType.Exp
                )
                vtmp = epi.tile([P, D], F32, tag="vtmp")
                nc.vector.tensor_add(vtmp, psv, bv_bc)
                nc.vector.tensor_mul(xt_tile[:, 0:D], vtmp, xt_tile[:, D : 2 * D])
                x_t.append(xt_tile)

            # ---- phase 2: per i-chunk big matmul + epilogue ----
            def epilogue(i, pd, pn, lo, hi):
                # out = num / (den * (1 + eq))  [sigmoid folded via eq=exp(-q)]
                w = hi - lo
                t1 = epi.tile([P, w], F32, tag="t1")
                nc.vector.tensor_mul(t1, pd, eq_t[i][:, lo:hi])
                nc.vector.tensor_add(t1, t1, pd)
                rec = epi.tile([P, w], F32, tag="rec")
                nc.vector.reciprocal(rec, t1)
                ob = epi.tile([P, w], F32, tag="ob")
                nc.vector.tensor_mul(ob, pn, rec)
                nc.sync.dma_start(out[i * P : (i + 1) * P, lo:hi], ob)

            GI = 4  # i-chunks per eb DMA group (1KB contiguous runs)
            eb_g = None
            for i in range(NCH):
                if i % GI == 0:
                    eb_g = ebpool.tile([P, NCH, GI * P], BF16, tag="eb")
                    nc.sync.dma_start(
                        eb_g, ebt_v[:, :, i * P : (i + GI) * P]
                    )
                eb = eb_g[:, :, (i % GI) * P : (i % GI + 1) * P]
                if i < NCH - 1:
                    pn = psum.tile([P, D], F32, tag="A", bufs=3)
                    pd = psum.tile([P, D], F32, tag="B", bufs=3)
                    for j in range(NCH):
                        nc.tensor.matmul(
                            pd, eb[:, j, :], x_t[j][:, D : 2 * D],
                            start=(j == 0), stop=(j == NCH - 1),
                        )
                        nc.tensor.matmul(
                            pn, eb[:, j, :], x_t[j][:, 0:D],
                            start=(j == 0), stop=(j == NCH - 1),
                        )
                    epilogue(i, pd, pn, 0, D)
                else:
                    # final chunk: split columns in half so the first half's
                    # epilogue (incl. the 3.4us reciprocal) overlaps the
                    # second half's matmuls instead of sitting in the tail
                    H = D // 2
                    for h in range(2):
                        # second half on otherwise-free tags: no slot contention
                        pn = psum.tile([P, H], F32, tag="A", bufs=3)
                        pd = psum.tile([P, H], F32, tag="B" if h == 0 else "C", bufs=3 if h == 0 else None)
                        for j in range(NCH):
                            nc.tensor.matmul(
                                pd, eb[:, j, :],
                                x_t[j][:, D + h * H : D + (h + 1) * H],
                                start=(j == 0), stop=(j == NCH - 1),
                            )
                            nc.tensor.matmul(
                                pn, eb[:, j, :], x_t[j][:, h * H : (h + 1) * H],
                                start=(j == 0), stop=(j == NCH - 1),
                            )
                        epilogue(i, pd, pn, h * H, (h + 1) * H)

    nc.compile()
    return nc


def get_nc():
    if "nc" not in _NC_CACHE:
        _NC_CACHE["nc"] = build_nc()
    return _NC_CACHE["nc"]


def prepare_in_maps(input, Wq, bq, Wk, bk, Wv, bv, pos_bias):
    wqkv = np.concatenate([Wq, Wk, Wv], axis=1).astype(NP_BF16)
    ebq = np.exp(-bq.astype(np.float64)).astype(np.float32)[None, :]
    bv2 = bv.astype(np.float32)[None, :]
    ebt = np.exp(np.ascontiguousarray(pos_bias.T)).astype(NP_BF16)
    in_maps = []
    for b in range(BS):
        xT = np.ascontiguousarray(input[b].T).astype(NP_BF16)
        in_maps.append(
            {"xT": xT, "wqkv": wqkv, "ebq": ebq, "bv": bv2, "ebt": ebt}
        )
    return in_maps


def kernel(input, Wq, bq, Wk, bk, Wv, bv, pos_bias, _run_kwargs=None):
    nc = get_nc()
    in_maps = prepare_in_maps(input, Wq, bq, Wk, bk, Wv, bv, pos_bias)
    res = run_bass_kernel_spmd(
        nc, in_maps, core_ids=list(range(BS)), **(_run_kwargs or {})
    )
    out = np.stack([res.results[b]["out"] for b in range(BS)], axis=0)
    if _run_kwargs:
        kernel.last_results = res
    return out
